# revision 5
# baseline (speedup 1.0000x reference)
"""Trainium2 Bass kernel: image -> additive-sinusoid audio encoding.

Math (per batch image b):
  gray = 255 * (w . rgb);  rev = flip(gray, rows);  avg = mean(gray)
  px   = clip(3*rev - 2*avg, 0, 255)
  A    = where(px==0, 0, exp(ln10 * (px/160 - 1.5)))            # [M=64 rows, N=64 cols]
  y[t] = sum_m A[m, col(t)] * sin(W[m]*t*dt + PHI0[m]),  col(t) = min(t//361, 63)
  audio= clip(0.5 + 2048*y, -32768, 32767)                       # [ns=23152]

Kernel strategy: t = n*361 + r  =>  angle = theta[i,n] + beta[i,r] (row flip folded
into the host tables), so  sinmat = sin(theta)cos(beta) + cos(theta)sin(beta).
Data-parallel over batch: 8 images per NeuronCore.

SBUF layout: partition p = (h, i) with h in {sin-half, cos-half} and i the image
row; the host ships gray255 duplicated onto both halves (so no on-device PE
duplication is needed), free = (b local image, n column).  Per image pair g:
  y_g[(b2,n), r] = sum_{(h,i)} PQ[(h,i),(b2,n)] * CS[(h,i),r]
with PQ = mask*Tt*E built as: mask*Tt runs on Pool in the shadow of the exp
activation, then one DVE multiply by E.  avg comes from a f16 row-reduce plus a
ones-matmul (cross-partition sum + broadcast in one shot); the ones matrix is
memset on device so the table DMA is off the critical path.  r runs to 409 so
the col-63 tail samples fall out of the same matmul.
Host side: input is pre-permuted grayscale in fp16; output comes back as fp16
y+0.5 and is clipped/cast during the unshard gather.
"""

import os

import numpy as np

# ---- problem constants (from the nn.Module definition; input-independent) ----
M = 64
N = 64
FL, FH, FS, T = 80.0, 7600.0, 22050, 1.05
NS = 2 * int(0.5 * FS * T)  # 23152
NUM = NS // N  # 361
RMAX = NS - (N - 1) * NUM  # 409 (last column's sample count)
DT = float(np.float32(1.0 / FS))  # reference rounds dt to f32 (jnp weak typing)
TWO_PI = 2.0 * np.pi
B = 64
N_CORES = 8
B_LOC = B // N_CORES  # 8 images per core
SCALE_SSM = (0.5 / np.sqrt(M)) * 32768.0  # 2048
LN10 = float(np.log(10.0))
EXP_A = LN10 / 160.0
EXP_B = -1.5 * LN10
W0, W1, W2 = 0.2989, 0.5870, 0.1140
ONES_VAL = 1.0 / 4096.0  # sum over 128 dup'd partitions -> 2*avg

# single table: [TT 512 | CS 409 | pad 1]
C_TT, C_CS = 0, 512
TABW = 512 + RMAX + 1  # 922, keeps row stride 4B-aligned


def _make_tables():
    # LCG phase bank (faithful port, ir starts at 0)
    ia, ic, im = 9301, 49297, 233280
    ir = 0
    phi = []
    for _ in range(M):
        ir = (ir * ia + ic) % im
        phi.append(TWO_PI * ir / im)
    phi32 = np.array(phi, np.float64).astype(np.float32)
    w32 = (TWO_PI * FL * (FH / FL) ** (np.arange(M) / (M - 1))).astype(np.float32)

    # fold the row flip (tf.reverse on axis 1) into the tables: row i uses W[63-i]
    wf = w32[::-1].astype(np.float64)
    phif = phi32[::-1].astype(np.float64)

    n_idx = np.arange(N, dtype=np.float64)
    theta = wf[:, None] * (n_idx[None, :] * NUM * DT) + phif[:, None]  # [64, 64]
    stct = np.concatenate([np.sin(theta), np.cos(theta)], axis=0)  # [128, 64]
    tt = np.tile(stct[:, None, :], (1, B_LOC, 1)).reshape(128, 512)

    r_idx = np.arange(RMAX, dtype=np.float64)
    beta = wf[:, None] * (r_idx[None, :] * DT)  # [64, 409]
    cs = np.concatenate(
        [SCALE_SSM * np.cos(beta), SCALE_SSM * np.sin(beta)], axis=0
    )  # [128, 409]

    pad = np.zeros((128, 1))
    tabs = np.concatenate([tt, cs, pad], axis=1).astype(np.float16)
    assert tabs.shape == (128, TABW), tabs.shape
    return {"tabs": np.ascontiguousarray(tabs)}


_TABLES = None


def tables():
    global _TABLES
    if _TABLES is None:
        _TABLES = _make_tables()
    return _TABLES


def build_nc():
    import concourse.bacc as bacc
    import concourse.bass as bass
    import concourse.mybir as mybir
    import concourse.tile as tile

    f32 = mybir.dt.float32
    f16 = mybir.dt.float16
    Alu = mybir.AluOpType
    Act = mybir.ActivationFunctionType

    nc = bacc.Bacc(
        "TRN2",
        target_bir_lowering=False,
        debug=False,
        num_devices=N_CORES,
        enable_asserts=False,
    )

    g16_d = nc.dram_tensor("g16", [128, 512], f16, kind="ExternalInput")
    tabs_d = nc.dram_tensor("tabs", [128, TABW], f16, kind="ExternalInput")
    audio_a_d = nc.dram_tensor("audio_a", [128, 2, RMAX], f16, kind="ExternalOutput")
    audio_b_d = nc.dram_tensor("audio_b", [128, 2, RMAX], f16, kind="ExternalOutput")

    with tile.TileContext(nc) as tc:
        with (
            tc.tile_pool(name="work", bufs=1) as work,
            tc.tile_pool(name="psum", bufs=1, space=bass.MemorySpace.PSUM) as psum,
        ):
            # device-built constants: ones matrix for the avg matmul (Pool),
            # Exp activation bias column (scalar engine requires an AP bias)
            ones = work.tile([128, 128], f16)
            nc.gpsimd.memset(ones, ONES_VAL)
            expb = work.tile([128, 1], f32)
            nc.vector.memset(expb, float(EXP_B))

            # ---- input DMAs on separate HWDGE rings ----
            G = work.tile([128, 512], f16)
            TB = work.tile([128, TABW], f16)
            nc.sync.dma_start(out=G, in_=g16_d[:])
            nc.gpsimd.dma_start(out=TB, in_=tabs_d[:])
            TT = TB[:, C_TT : C_TT + 512]
            CS = TB[:, C_CS : C_CS + RMAX]

            # ---- mean path: f16 row-reduce, then one ones-matmul that both
            # reduces across partitions and broadcasts 2*avg everywhere ----
            rs16 = work.tile([128, B_LOC], f16)
            with nc.allow_low_precision("f16 rowsum of <=64 uint8-scale values"):
                nc.vector.reduce_sum(
                    out=rs16, in_=G.rearrange("p (b n) -> p b n", b=B_LOC),
                    axis=mybir.AxisListType.X,
                )
            csS2 = psum.tile([128, B_LOC], f32)
            nc.tensor.matmul(csS2, ones, rs16, start=True, stop=True)

            # ---- px -> clip -> exp (per image-half for pipelining) ----
            Gb = G.rearrange("p (b n) -> p b n", b=B_LOC)
            px = work.tile([128, B_LOC, 64], f32)
            E = work.tile([128, 512], f16)
            A = work.tile([128, 512], f16)
            PQ = work.tile([128, 512], f16)
            H = B_LOC // 2  # 4 images per half
            for s in range(2):
                sl = slice(s * H, (s + 1) * H)
                nc.vector.scalar_tensor_tensor(
                    out=px[:, sl], in0=Gb[:, sl], scalar=3.0,
                    in1=csS2[:, sl].broadcast_to([128, H, 64]),
                    op0=Alu.mult, op1=Alu.subtract,
                )
                nc.vector.tensor_scalar(
                    out=px[:, sl], in0=px[:, sl], scalar1=0.0, scalar2=255.0,
                    op0=Alu.max, op1=Alu.min,
                )
            for s in range(2):
                sl = slice(s * H, (s + 1) * H)
                fl = slice(s * 256, (s + 1) * 256)
                nc.scalar.activation(
                    out=E[:, fl], in_=px[:, sl].rearrange("p a b -> p (a b)"),
                    func=Act.Exp, bias=expb, scale=float(EXP_A),
                )
            for s in range(2):
                sl = slice(s * H, (s + 1) * H)
                fl = slice(s * 256, (s + 1) * 256)
                nc.vector.scalar_tensor_tensor(
                    out=A[:, fl], in0=px[:, sl].rearrange("p a b -> p (a b)"),
                    scalar=0.0, in1=E[:, fl], op0=Alu.is_gt, op1=Alu.mult,
                )
                nc.vector.tensor_mul(out=PQ[:, fl], in0=A[:, fl], in1=TT[:, fl])

            # ---- one K=128 N=409 matmul per image pair, split drains, and a
            # per-pair output DMA alternating between the SWDGE and SP rings ----
            Ua = work.tile([128, 2, RMAX], f16)
            Ub = work.tile([128, 2, RMAX], f16)
            HCUT = 208
            for g in range(4):
                yt = psum.tile([128, RMAX], f32, name=f"y{g}")
                nc.tensor.matmul(
                    yt, PQ[:, 128 * g : 128 * (g + 1)], CS, start=True, stop=True
                )
                U, D, k = (Ua, audio_a_d, g) if g < 2 else (Ub, audio_b_d, g - 2)
                nc.vector.tensor_scalar(
                    out=U[:, k, 0:HCUT], in0=yt[:, 0:HCUT],
                    scalar1=0.5, scalar2=0.0, op0=Alu.add, op1=Alu.bypass,
                )
                nc.scalar.activation(
                    out=U[:, k, HCUT:RMAX], in_=yt[:, HCUT:RMAX],
                    func=Act.Copy, bias=0.5, scale=1.0,
                )
                eng = nc.gpsimd if g % 2 == 0 else nc.sync
                eng.dma_start(out=D[:, k], in_=U[:, k])

    nc.compile()
    return nc


_NC = None


def _get_nc():
    global _NC
    if _NC is None:
        _NC = build_nc()
    return _NC


LAST_RESULTS = None


def kernel(x: np.ndarray) -> np.ndarray:
    from concourse.bass_utils import run_bass_kernel_spmd

    x = np.asarray(x, dtype=np.float32)
    assert x.shape == (B, 64, 64, 3), x.shape

    # shard + permute to the SBUF layout [p=(h,i), (b, n)], fp16 grayscale,
    # duplicated across the two partition halves (sin/cos banks)
    gray = 255.0 * (x[..., 0] * W0 + x[..., 1] * W1 + x[..., 2] * W2)  # [B,64,64]
    gc = gray.reshape(N_CORES, B_LOC, 64, 64)  # [core, b, i, n]
    g1 = np.ascontiguousarray(gc.transpose(0, 2, 1, 3)).reshape(N_CORES, 64, 512)
    g16 = np.tile(g1, (1, 2, 1)).astype(np.float16)  # [core, 128, 512]

    nc = _get_nc()
    tbl = tables()
    in_maps = []
    for c in range(N_CORES):
        m = {"g16": np.ascontiguousarray(g16[c])}
        m.update(tbl)
        in_maps.append(m)

    trace = os.environ.get("BASS_KERNEL_TRACE", "0") == "1"
    res = run_bass_kernel_spmd(
        nc, in_maps, core_ids=list(range(N_CORES)), trace=trace
    )
    global LAST_RESULTS
    LAST_RESULTS = res

    outs = np.empty((B, NS), np.float32)
    for c, r in enumerate(res.results):
        for half, name in enumerate(("audio_a", "audio_b")):
            # [p=(b2,n), k=g-in-half, r]
            arr = r[name].astype(np.float32).reshape(2, 64, 2, RMAX)
            for k in range(2):
                g = 2 * half + k
                for b2 in range(2):
                    img = arr[b2, :, k]  # [64 cols, RMAX]
                    row = c * B_LOC + 2 * g + b2
                    outs[row, : N * NUM] = img[:, :NUM].reshape(N * NUM)
                    outs[row, N * NUM :] = img[63, NUM:]
    np.clip(outs, -32768.0, 32767.0, out=outs)
    return outs


# revision 9
# speedup vs baseline: 1.1302x; 1.1302x over previous
"""Trainium2 Bass kernel: image -> additive-sinusoid audio encoding.

Math (per batch image b):
  gray = 255 * (w . rgb);  rev = flip(gray, rows);  avg = mean(gray)
  px   = clip(3*rev - 2*avg, 0, 255)
  A    = where(px==0, 0, exp(ln10 * (px/160 - 1.5)))            # [M=64 rows, N=64 cols]
  y[t] = sum_m A[m, col(t)] * sin(W[m]*t*dt + PHI0[m]),  col(t) = min(t//361, 63)
  audio= clip(0.5 + 2048*y, -32768, 32767)                       # [ns=23152]

Kernel strategy: t = n*361 + r  =>  angle = theta[i,n] + beta[i,r] (row flip folded
into the host tables), so  sinmat = sin(theta)cos(beta) + cos(theta)sin(beta).
Data-parallel over batch: 8 images per NeuronCore.

SBUF layout: partition p = (h, i) with h in {sin-half, cos-half} and i the image
row; the host ships gray255 duplicated onto both halves (so no on-device PE
duplication is needed), free = (b local image, n column).  Per image pair g:
  y_g[(b2,n), r] = sum_{(h,i)} PQ[(h,i),(b2,n)] * CS[(h,i),r]
with PQ = mask*Tt*E built as: mask*Tt runs on Pool in the shadow of the exp
activation, then one DVE multiply by E.  avg comes from a f16 row-reduce plus a
ones-matmul (cross-partition sum + broadcast in one shot); the ones matrix is
memset on device so the table DMA is off the critical path.  r runs to 409 so
the col-63 tail samples fall out of the same matmul.
Host side: input is pre-permuted grayscale in fp16; output comes back as fp16
y+0.5 and is clipped/cast during the unshard gather.
"""

import os

import numpy as np

# ---- problem constants (from the nn.Module definition; input-independent) ----
M = 64
N = 64
FL, FH, FS, T = 80.0, 7600.0, 22050, 1.05
NS = 2 * int(0.5 * FS * T)  # 23152
NUM = NS // N  # 361
RMAX = NS - (N - 1) * NUM  # 409 (last column's sample count)
DT = float(np.float32(1.0 / FS))  # reference rounds dt to f32 (jnp weak typing)
TWO_PI = 2.0 * np.pi
B = 64
N_CORES = 8
B_LOC = B // N_CORES  # 8 images per core
SCALE_SSM = (0.5 / np.sqrt(M)) * 32768.0  # 2048
LN10 = float(np.log(10.0))
EXP_A = LN10 / 160.0
EXP_B = -1.5 * LN10
W0, W1, W2 = 0.2989, 0.5870, 0.1140
ONES_VAL = 1.0 / 4096.0  # sum over 128 dup'd partitions -> 2*avg

# single table: [TT 512 | CS 409 | pad 1]
C_TT, C_CS = 0, 512
TABW = 512 + RMAX + 1  # 922, keeps row stride 4B-aligned


def _make_tables():
    # LCG phase bank (faithful port, ir starts at 0)
    ia, ic, im = 9301, 49297, 233280
    ir = 0
    phi = []
    for _ in range(M):
        ir = (ir * ia + ic) % im
        phi.append(TWO_PI * ir / im)
    phi32 = np.array(phi, np.float64).astype(np.float32)
    w32 = (TWO_PI * FL * (FH / FL) ** (np.arange(M) / (M - 1))).astype(np.float32)

    # fold the row flip (tf.reverse on axis 1) into the tables: row i uses W[63-i]
    wf = w32[::-1].astype(np.float64)
    phif = phi32[::-1].astype(np.float64)

    n_idx = np.arange(N, dtype=np.float64)
    theta = wf[:, None] * (n_idx[None, :] * NUM * DT) + phif[:, None]  # [64, 64]
    stct = np.concatenate([np.sin(theta), np.cos(theta)], axis=0)  # [128, 64]
    tt = np.tile(stct[:, None, :], (1, B_LOC, 1)).reshape(128, 512)

    r_idx = np.arange(RMAX, dtype=np.float64)
    beta = wf[:, None] * (r_idx[None, :] * DT)  # [64, 409]
    cs = np.concatenate(
        [SCALE_SSM * np.cos(beta), SCALE_SSM * np.sin(beta)], axis=0
    )  # [128, 409]

    pad = np.zeros((128, 1))
    tabs = np.concatenate([tt, cs, pad], axis=1).astype(np.float16)
    assert tabs.shape == (128, TABW), tabs.shape
    return {"tabs": np.ascontiguousarray(tabs)}


_TABLES = None


def tables():
    global _TABLES
    if _TABLES is None:
        _TABLES = _make_tables()
    return _TABLES


def build_nc():
    import concourse.bacc as bacc
    import concourse.bass as bass
    import concourse.mybir as mybir
    import concourse.tile as tile

    f32 = mybir.dt.float32
    f16 = mybir.dt.float16
    Alu = mybir.AluOpType
    Act = mybir.ActivationFunctionType

    nc = bacc.Bacc(
        "TRN2",
        target_bir_lowering=False,
        debug=False,
        num_devices=N_CORES,
        enable_asserts=False,
    )

    g16_d = nc.dram_tensor("g16", [128, 512], f16, kind="ExternalInput")
    tabs_d = nc.dram_tensor("tabs", [128, TABW], f16, kind="ExternalInput")
    audio_a_d = nc.dram_tensor("audio_a", [128, 2, RMAX], f16, kind="ExternalOutput")
    audio_b_d = nc.dram_tensor("audio_b", [128, 2, RMAX], f16, kind="ExternalOutput")

    with tile.TileContext(nc) as tc:
        with (
            tc.tile_pool(name="work", bufs=1) as work,
            tc.tile_pool(name="psum", bufs=1, space=bass.MemorySpace.PSUM) as psum,
        ):
            # device-built constants: ones matrix for the avg matmul (Pool),
            # Exp activation bias column (scalar engine requires an AP bias)
            ones = work.tile([128, 128], f16)
            nc.gpsimd.memset(ones, ONES_VAL)
            expb = work.tile([128, 1], f32)
            nc.vector.memset(expb, float(EXP_B))

            # ---- input DMAs on separate HWDGE rings ----
            G = work.tile([128, 512], f16)
            TB = work.tile([128, TABW], f16)
            nc.sync.dma_start(out=G, in_=g16_d[:])
            nc.sync.dma_start(out=TB, in_=tabs_d[:])
            TT = TB[:, C_TT : C_TT + 512]
            CS = TB[:, C_CS : C_CS + RMAX]

            # ---- mean path: f16 row-reduce, then one ones-matmul that both
            # reduces across partitions and broadcasts 2*avg everywhere ----
            rs16 = work.tile([128, B_LOC], f16)
            with nc.allow_low_precision("f16 rowsum of <=64 uint8-scale values"):
                nc.vector.reduce_sum(
                    out=rs16, in_=G.rearrange("p (b n) -> p b n", b=B_LOC),
                    axis=mybir.AxisListType.X,
                )
            csS2 = psum.tile([128, B_LOC], f32)
            nc.tensor.matmul(csS2, ones, rs16, start=True, stop=True)
            # copy 2*avg to SBUF f16 so the px stt runs in DVE 16-bit mode
            cs16 = work.tile([128, B_LOC], f16)
            nc.vector.tensor_scalar(
                out=cs16, in0=csS2, scalar1=0.0, scalar2=0.0,
                op0=Alu.add, op1=Alu.bypass,
            )

            # ---- px -> clip -> exp (per image-half for pipelining) ----
            Gb = G.rearrange("p (b n) -> p b n", b=B_LOC)
            px = work.tile([128, B_LOC, 64], f16)
            E = work.tile([128, 512], f16)
            A = work.tile([128, 512], f16)
            PQ = work.tile([128, 512], f16)
            H = B_LOC // 2  # 4 images per half
            for s in range(2):
                sl = slice(s * H, (s + 1) * H)
                nc.vector.scalar_tensor_tensor(
                    out=px[:, sl], in0=Gb[:, sl], scalar=3.0,
                    in1=cs16[:, sl].broadcast_to([128, H, 64]),
                    op0=Alu.mult, op1=Alu.subtract,
                )
                nc.vector.tensor_scalar(
                    out=px[:, sl], in0=px[:, sl], scalar1=0.0, scalar2=255.0,
                    op0=Alu.max, op1=Alu.min,
                )
            for s in range(2):
                sl = slice(s * H, (s + 1) * H)
                fl = slice(s * 256, (s + 1) * 256)
                nc.scalar.activation(
                    out=E[:, fl], in_=px[:, sl].rearrange("p a b -> p (a b)"),
                    func=Act.Exp, bias=expb, scale=float(EXP_A),
                )
            for s in range(2):
                sl = slice(s * H, (s + 1) * H)
                fl = slice(s * 256, (s + 1) * 256)
                nc.vector.scalar_tensor_tensor(
                    out=A[:, fl], in0=px[:, sl].rearrange("p a b -> p (a b)"),
                    scalar=0.0, in1=E[:, fl], op0=Alu.is_gt, op1=Alu.mult,
                )
                nc.vector.tensor_mul(out=PQ[:, fl], in0=A[:, fl], in1=TT[:, fl])

            # ---- one K=128 N=409 matmul per image pair, split drains, and a
            # per-pair output DMA alternating between the SWDGE and SP rings ----
            Ua = work.tile([128, 2, RMAX], f16)
            Ub = work.tile([128, 2, RMAX], f16)
            HCUT = 208
            for g in range(4):
                yt = psum.tile([128, RMAX], f32, name=f"y{g}")
                nc.tensor.matmul(
                    yt, PQ[:, 128 * g : 128 * (g + 1)], CS, start=True, stop=True
                )
                U, k = (Ua, g) if g < 2 else (Ub, g - 2)
                nc.vector.tensor_scalar(
                    out=U[:, k, 0:HCUT], in0=yt[:, 0:HCUT],
                    scalar1=0.5, scalar2=0.0, op0=Alu.add, op1=Alu.bypass,
                )
                nc.scalar.activation(
                    out=U[:, k, HCUT:RMAX], in_=yt[:, HCUT:RMAX],
                    func=Act.Copy, bias=0.5, scale=1.0,
                )
                if g == 1:
                    nc.sync.dma_start(out=audio_a_d[:], in_=Ua)
            nc.sync.dma_start(out=audio_b_d[:], in_=Ub)

    nc.compile()
    return nc


_NC = None


def _get_nc():
    global _NC
    if _NC is None:
        _NC = build_nc()
    return _NC


LAST_RESULTS = None


def kernel(x: np.ndarray) -> np.ndarray:
    from concourse.bass_utils import run_bass_kernel_spmd

    x = np.asarray(x, dtype=np.float32)
    assert x.shape == (B, 64, 64, 3), x.shape

    # shard + permute to the SBUF layout [p=(h,i), (b, n)], fp16 grayscale,
    # duplicated across the two partition halves (sin/cos banks)
    gray = 255.0 * (x[..., 0] * W0 + x[..., 1] * W1 + x[..., 2] * W2)  # [B,64,64]
    gc = gray.reshape(N_CORES, B_LOC, 64, 64)  # [core, b, i, n]
    g1 = np.ascontiguousarray(gc.transpose(0, 2, 1, 3)).reshape(N_CORES, 64, 512)
    g16 = np.tile(g1, (1, 2, 1)).astype(np.float16)  # [core, 128, 512]

    nc = _get_nc()
    tbl = tables()
    in_maps = []
    for c in range(N_CORES):
        m = {"g16": np.ascontiguousarray(g16[c])}
        m.update(tbl)
        in_maps.append(m)

    trace = os.environ.get("BASS_KERNEL_TRACE", "0") == "1"
    res = run_bass_kernel_spmd(
        nc, in_maps, core_ids=list(range(N_CORES)), trace=trace
    )
    global LAST_RESULTS
    LAST_RESULTS = res

    outs = np.empty((B, NS), np.float32)
    for c, r in enumerate(res.results):
        for half, name in enumerate(("audio_a", "audio_b")):
            # [p=(b2,n), k=g-in-half, r]
            arr = r[name].astype(np.float32).reshape(2, 64, 2, RMAX)
            for k in range(2):
                g = 2 * half + k
                for b2 in range(2):
                    img = arr[b2, :, k]  # [64 cols, RMAX]
                    row = c * B_LOC + 2 * g + b2
                    outs[row, : N * NUM] = img[:, :NUM].reshape(N * NUM)
                    outs[row, N * NUM :] = img[63, NUM:]
    np.clip(outs, -32768.0, 32767.0, out=outs)
    return outs


# revision 13
# speedup vs baseline: 1.1687x; 1.0341x over previous
"""Trainium2 Bass kernel: image -> additive-sinusoid audio encoding.

Math (per batch image b):
  gray = 255 * (w . rgb);  rev = flip(gray, rows);  avg = mean(gray)
  px   = clip(3*rev - 2*avg, 0, 255)
  A    = where(px==0, 0, exp(ln10 * (px/160 - 1.5)))            # [M=64 rows, N=64 cols]
  y[t] = sum_m A[m, col(t)] * sin(W[m]*t*dt + PHI0[m]),  col(t) = min(t//361, 63)
  audio= clip(0.5 + 2048*y, -32768, 32767)                       # [ns=23152]

Kernel strategy: t = n*361 + r  =>  angle = theta[i,n] + beta[i,r] (row flip folded
into the host tables), so  sinmat = sin(theta)cos(beta) + cos(theta)sin(beta).
Data-parallel over batch: 8 images per NeuronCore.

SBUF layout: partition p = (h, i) with h in {sin-half, cos-half} and i the image
row; the host ships gray255 duplicated onto both halves (so no on-device PE
duplication is needed), free = (b local image, n column).  Per image pair g:
  y_g[(b2,n), r] = sum_{(h,i)} PQ[(h,i),(b2,n)] * CS[(h,i),r]
with PQ = mask*Tt*E built as: mask*Tt runs on Pool in the shadow of the exp
activation, then one DVE multiply by E.  avg comes from a f16 row-reduce plus a
ones-matmul (cross-partition sum + broadcast in one shot); the ones matrix is
memset on device so the table DMA is off the critical path.  r runs to 409 so
the col-63 tail samples fall out of the same matmul.
Host side: input is pre-permuted grayscale in fp16; output comes back as fp16
y+0.5 and is clipped/cast during the unshard gather.
"""

import os

import numpy as np

# ---- problem constants (from the nn.Module definition; input-independent) ----
M = 64
N = 64
FL, FH, FS, T = 80.0, 7600.0, 22050, 1.05
NS = 2 * int(0.5 * FS * T)  # 23152
NUM = NS // N  # 361
RMAX = NS - (N - 1) * NUM  # 409 (last column's sample count)
DT = float(np.float32(1.0 / FS))  # reference rounds dt to f32 (jnp weak typing)
TWO_PI = 2.0 * np.pi
B = 64
N_CORES = 8
B_LOC = B // N_CORES  # 8 images per core
SCALE_SSM = (0.5 / np.sqrt(M)) * 32768.0  # 2048
LN10 = float(np.log(10.0))
EXP_A = LN10 / 160.0
EXP_B = -1.5 * LN10
W0, W1, W2 = 0.2989, 0.5870, 0.1140
ONES_VAL = 1.0 / (3.0 * 4096.0)  # 128 dup'd partitions of 3*gray -> 2*avg

# single table: [TT 512 | CS 409 | pad 1]
C_TT, C_CS = 0, 512
TABW = 512 + RMAX + 1  # 922, keeps row stride 4B-aligned


def _make_tables():
    # LCG phase bank (faithful port, ir starts at 0)
    ia, ic, im = 9301, 49297, 233280
    ir = 0
    phi = []
    for _ in range(M):
        ir = (ir * ia + ic) % im
        phi.append(TWO_PI * ir / im)
    phi32 = np.array(phi, np.float64).astype(np.float32)
    w32 = (TWO_PI * FL * (FH / FL) ** (np.arange(M) / (M - 1))).astype(np.float32)

    # fold the row flip (tf.reverse on axis 1) into the tables: row i uses W[63-i]
    wf = w32[::-1].astype(np.float64)
    phif = phi32[::-1].astype(np.float64)

    n_idx = np.arange(N, dtype=np.float64)
    theta = wf[:, None] * (n_idx[None, :] * NUM * DT) + phif[:, None]  # [64, 64]
    stct = np.concatenate([np.sin(theta), np.cos(theta)], axis=0)  # [128, 64]
    tt = np.tile(stct[:, None, :], (1, B_LOC, 1)).reshape(128, 512)

    r_idx = np.arange(RMAX, dtype=np.float64)
    beta = wf[:, None] * (r_idx[None, :] * DT)  # [64, 409]
    cs = np.concatenate(
        [SCALE_SSM * np.cos(beta), SCALE_SSM * np.sin(beta)], axis=0
    )  # [128, 409]

    pad = np.zeros((128, 1))
    tabs = np.concatenate([tt, cs, pad], axis=1).astype(np.float16)
    assert tabs.shape == (128, TABW), tabs.shape
    return {"tabs": np.ascontiguousarray(tabs)}


_TABLES = None


def tables():
    global _TABLES
    if _TABLES is None:
        _TABLES = _make_tables()
    return _TABLES


def build_nc():
    import concourse.bacc as bacc
    import concourse.bass as bass
    import concourse.mybir as mybir
    import concourse.tile as tile

    f32 = mybir.dt.float32
    f16 = mybir.dt.float16
    Alu = mybir.AluOpType
    Act = mybir.ActivationFunctionType

    nc = bacc.Bacc(
        "TRN2",
        target_bir_lowering=False,
        debug=False,
        num_devices=N_CORES,
        enable_asserts=False,
    )

    g16_d = nc.dram_tensor("g16", [128, 512], f16, kind="ExternalInput")
    tabs_d = nc.dram_tensor("tabs", [128, TABW], f16, kind="ExternalInput")
    audio_a_d = nc.dram_tensor("audio_a", [128, 2, RMAX], f16, kind="ExternalOutput")
    audio_b_d = nc.dram_tensor("audio_b", [128, 2, RMAX], f16, kind="ExternalOutput")

    with tile.TileContext(nc) as tc:
        with (
            tc.tile_pool(name="work", bufs=1) as work,
            tc.tile_pool(name="psum", bufs=1, space=bass.MemorySpace.PSUM) as psum,
        ):
            # device-built constants: ones matrix for the avg matmul (Pool),
            # Exp activation bias column (scalar engine requires an AP bias)
            ones = work.tile([128, 128], f16)
            nc.gpsimd.memset(ones, ONES_VAL)
            expb = work.tile([128, 1], f32)
            nc.vector.memset(expb, float(EXP_B))

            # ---- input DMAs on separate HWDGE rings ----
            G = work.tile([128, 512], f16)
            TB = work.tile([128, TABW], f16)
            nc.sync.dma_start(out=G, in_=g16_d[:])
            nc.sync.dma_start(out=TB, in_=tabs_d[:])
            TT = TB[:, C_TT : C_TT + 512]
            CS = TB[:, C_CS : C_CS + RMAX]

            # ---- mean path, split per image-half: f16 row-reduce, then a
            # ones-matmul per half that both reduces across partitions and
            # broadcasts 2*avg everywhere (half 0 unblocks px0 early) ----
            H = B_LOC // 2  # 4 images per half
            Gb = G.rearrange("p (b n) -> p b n", b=B_LOC)
            rs16 = work.tile([128, B_LOC], f16)
            cs2 = []
            with nc.allow_low_precision("f16 rowsum of <=64 uint8-scale values"):
                for s in range(2):
                    sl = slice(s * H, (s + 1) * H)
                    nc.vector.reduce_sum(
                        out=rs16[:, sl], in_=Gb[:, sl], axis=mybir.AxisListType.X,
                    )
                    ct = psum.tile([128, H], f32, name=f"cs{s}")
                    nc.tensor.matmul(ct, ones, rs16[:, sl], start=True, stop=True)
                    cs2.append(ct)

            # ---- px = G3 - 2*avg -> min(255) -> exp; mask -> PQ ----
            px = work.tile([128, B_LOC, 64], f16)
            E = work.tile([128, 512], f16)
            A = work.tile([128, 512], f16)
            PQ = work.tile([128, 512], f16)
            for s in range(2):
                sl = slice(s * H, (s + 1) * H)
                nc.vector.tensor_sub(
                    out=px[:, sl], in0=Gb[:, sl],
                    in1=cs2[s].broadcast_to([128, H, 64]),
                )
                nc.vector.tensor_scalar(
                    out=px[:, sl], in0=px[:, sl], scalar1=255.0, scalar2=0.0,
                    op0=Alu.min, op1=Alu.bypass,
                )
            for s in range(2):
                sl = slice(s * H, (s + 1) * H)
                fl = slice(s * 256, (s + 1) * 256)
                nc.scalar.activation(
                    out=E[:, fl], in_=px[:, sl].rearrange("p a b -> p (a b)"),
                    func=Act.Exp, bias=expb, scale=float(EXP_A),
                )
            for s in range(2):
                sl = slice(s * H, (s + 1) * H)
                fl = slice(s * 256, (s + 1) * 256)
                nc.vector.scalar_tensor_tensor(
                    out=A[:, fl], in0=px[:, sl].rearrange("p a b -> p (a b)"),
                    scalar=0.0, in1=E[:, fl], op0=Alu.is_gt, op1=Alu.mult,
                )
                nc.vector.tensor_mul(out=PQ[:, fl], in0=A[:, fl], in1=TT[:, fl])

            # ---- one K=128 N=409 matmul per image pair, split drains, and a
            # per-pair output DMA alternating between the SWDGE and SP rings ----
            Ua = work.tile([128, 2, RMAX], f16)
            Ub = work.tile([128, 2, RMAX], f16)
            HCUT = 208
            for g in range(4):
                yt = psum.tile([128, RMAX], f32, name=f"y{g}")
                nc.tensor.matmul(
                    yt, PQ[:, 128 * g : 128 * (g + 1)], CS, start=True, stop=True
                )
                U, k = (Ua, g) if g < 2 else (Ub, g - 2)
                nc.vector.tensor_scalar(
                    out=U[:, k, 0:HCUT], in0=yt[:, 0:HCUT],
                    scalar1=0.5, scalar2=0.0, op0=Alu.add, op1=Alu.bypass,
                )
                nc.scalar.activation(
                    out=U[:, k, HCUT:RMAX], in_=yt[:, HCUT:RMAX],
                    func=Act.Copy, bias=0.5, scale=1.0,
                )
                if g == 1:
                    nc.sync.dma_start(out=audio_a_d[:], in_=Ua)
            nc.scalar.dma_start(out=audio_b_d[:], in_=Ub)

    nc.compile()
    return nc


_NC = None


def _get_nc():
    global _NC
    if _NC is None:
        _NC = build_nc()
    return _NC


LAST_RESULTS = None


def kernel(x: np.ndarray) -> np.ndarray:
    from concourse.bass_utils import run_bass_kernel_spmd

    x = np.asarray(x, dtype=np.float32)
    assert x.shape == (B, 64, 64, 3), x.shape

    # shard + permute to the SBUF layout [p=(h,i), (b, n)], fp16 grayscale,
    # duplicated across the two partition halves (sin/cos banks)
    # the contrast-stretch 3x is folded into the host grayscale (px = G3 - 2*avg)
    gray = 765.0 * (x[..., 0] * W0 + x[..., 1] * W1 + x[..., 2] * W2)  # [B,64,64]
    gc = gray.reshape(N_CORES, B_LOC, 64, 64)  # [core, b, i, n]
    g1 = np.ascontiguousarray(gc.transpose(0, 2, 1, 3)).reshape(N_CORES, 64, 512)
    g16 = np.tile(g1, (1, 2, 1)).astype(np.float16)  # [core, 128, 512]

    nc = _get_nc()
    tbl = tables()
    in_maps = []
    for c in range(N_CORES):
        m = {"g16": np.ascontiguousarray(g16[c])}
        m.update(tbl)
        in_maps.append(m)

    trace = os.environ.get("BASS_KERNEL_TRACE", "0") == "1"
    res = run_bass_kernel_spmd(
        nc, in_maps, core_ids=list(range(N_CORES)), trace=trace
    )
    global LAST_RESULTS
    LAST_RESULTS = res

    outs = np.empty((B, NS), np.float32)
    for c, r in enumerate(res.results):
        for half, name in enumerate(("audio_a", "audio_b")):
            # [p=(b2,n), k=g-in-half, r]
            arr = r[name].astype(np.float32).reshape(2, 64, 2, RMAX)
            for k in range(2):
                g = 2 * half + k
                for b2 in range(2):
                    img = arr[b2, :, k]  # [64 cols, RMAX]
                    row = c * B_LOC + 2 * g + b2
                    outs[row, : N * NUM] = img[:, :NUM].reshape(N * NUM)
                    outs[row, N * NUM :] = img[63, NUM:]
    np.clip(outs, -32768.0, 32767.0, out=outs)
    return outs
